# revision 13
# baseline (speedup 1.0000x reference)
"""Fused multi-head attention layer for Trainium2, 8-core data-parallel.

Problem: x[8,1024,768] -> qkv proj (w_qkv[2304,768]) -> 12-head attention
(head_dim 64, key-padding mask) -> out proj (w_proj[768,768] + b_proj).

Strategy:
  * Data parallel over batch: core b handles x[b] end to end. No collectives.
  * Host pre-transposes x / w_qkv / w_proj so every device matmul is
    native-layout (contraction dim on partitions): xT[d,l], w1T[d,e],
    w2T[din,dout] (+bias row).
  * QK^T is computed as qkvT[e,l] (e on partitions) so per-head Q^T/K^T
    [64,1024] slices are direct matmul operands; V is computed un-transposed
    [l, 768] so V'[m, 65] slices (with a ones column) are AV lhsT operands.
  * Scores are computed TRANSPOSED: S.T[m,l] = K @ Q.T. Softmax runs without
    max-subtraction (scores are O(1) by construction: x~N(0,1), w~N(0,.02^2)),
    so exp is a single scalar-engine activation with the key-padding mask
    folded in as a per-partition additive bias and the 1/sqrt(hd) scale folded
    into the activation scale. exp(S.T) is then directly the rhs of the AV
    matmul -- no P transpose anywhere.
  * The softmax denominator comes free from a ones column appended to V
    (row 64 of the AV accumulator). O' is staged to SBUF immediately (2 DVE
    copies) so the PSUM accumulator frees fast; normalization (DVE
    reciprocal-approx + GPSIMD partition_broadcast + DVE multiply) runs off
    the critical path, overlapped with the next head pair.
  * All matmuls use float32r (full fp32 data, 1 cycle/row on TRN2 for free
    dim >= 256) -- fp32 accuracy at bf16 speed.
  * PSUM->SBUF evacuation of the projection phases runs on the scalar engine
    (idle there), keeping DVE for the attention epilogue.
"""

import os
import sys

import numpy as np

sys.path.insert(0, "/opt/trn_rl_repo")

B, L, D, H, HD = 8, 1024, 768, 12, 64
E = 3 * D
SCALE = HD ** -0.5
P = 128
KC = D // P          # 6 contraction chunks of 128 over d
LT = L // P          # 8 l/m partition tiles
NP = H // 2          # 6 head pairs
NCORES = 8
NEG = -30000.0       # mask bias; exp(NEG + s) == 0 in fp32

_cached = {}


def _build_program():
    import concourse.tile as tile
    from concourse import bacc, mybir

    f32 = mybir.dt.float32
    f32r = mybir.dt.float32r
    AF = mybir.ActivationFunctionType

    nc = bacc.Bacc(trn_type="TRN2", target_bir_lowering=False, debug=False)

    xT_d = nc.declare_dram_parameter("xT", [D, L], f32r, isOutput=False)
    w1T_d = nc.declare_dram_parameter("w1T", [D, E], f32r, isOutput=False)
    w2Tb_d = nc.declare_dram_parameter("w2Tb", [D + 1, D], f32r, isOutput=False)
    mbias_d = nc.declare_dram_parameter("mbias", [L], f32, isOutput=False)
    ones_d = nc.declare_dram_parameter("ones", [P, H + 1], f32r, isOutput=False)
    out_d = nc.declare_dram_parameter("out", [L, D], f32, isOutput=True)

    def r(ap):
        return ap

    with tile.TileContext(nc) as tc:
        from contextlib import ExitStack

        with ExitStack() as ctx:
            persist = ctx.enter_context(tc.tile_pool(name="persist", bufs=1))
            # qkvT for Q and K: e-tiles 0..5 = Q heads (2 per tile), 6..11 = K
            qkT_sb = persist.tile([P, 2 * KC, L], f32r)
            # V with a ones column per head: [l-tile, head, 65]
            V_sb = persist.tile([P, LT, H * (HD + 1)], f32r)
            V_v = V_sb[:].rearrange("p l (h c) -> p l h c", c=HD + 1)
            OT_sb = persist.tile([P, KC, L], f32r)       # O.T, heads stacked
            bias_sb = persist.tile([P, LT], f32)        # mask bias per key pos
            ones_sb = persist.tile([1, P], f32r)

            nc.sync.dma_start(
                out=ones_sb[0:1, :],
                in_=ones_d.ap().rearrange("p h -> (p h)")[None, 0:P],
            )
            for j in range(LT):
                nc.gpsimd.dma_start(
                    out=V_v[:, j, :, HD], in_=ones_d[:, 0:H]
                )
            nc.sync.dma_start(
                out=bias_sb[:], in_=mbias_d.ap().rearrange("(o p) -> p o", p=P)
            )

            # ---------------- Phase A: QKV projection ----------------
            with tc.tile_pool(name="phA", bufs=1) as pA, tc.tile_pool(
                name="psA", bufs=2, space="PSUM"
            ) as psA:
                xT_sb = pA.tile([P, KC, L], f32r)
                w1T_sb = pA.tile([P, KC, E], f32r)
                xT_r = xT_d.ap().rearrange("(o p) l -> p o l", p=P)
                w1T_r = w1T_d.ap().rearrange("(o p) e -> p o e", p=P)
                # chunked loads so the first matmuls start early
                for k0 in range(0, KC, 3):
                    nc.sync.dma_start(
                        out=xT_sb[:, k0 : k0 + 3, :], in_=xT_r[:, k0 : k0 + 3, :]
                    )
                EW = 576
                for e0 in range(0, E, EW):
                    nc.sync.dma_start(
                        out=w1T_sb[:, :, e0 : e0 + EW],
                        in_=w1T_r[:, :, e0 : e0 + EW],
                    )

                # qkT[e,l] = w1.T.T @ xT for e in [0, 1536)
                for et in range(2 * KC):
                    ps = psA.tile([P, L], f32, tag="qk")
                    for c in range(2):
                        for k in range(KC):
                            nc.tensor.matmul(
                                ps[:, c * 512 : (c + 1) * 512],
                                lhsT=r(w1T_sb[:, k, et * P : (et + 1) * P]),
                                rhs=r(xT_sb[:, k, c * 512 : (c + 1) * 512]),
                                start=(k == 0),
                                stop=(k == KC - 1),
                            )
                    nc.scalar.copy(qkT_sb[:, et, :], ps[:])

                # V[l, dv] = x @ w1_v.T  (dv in [1536, 2304))
                for i in range(LT):
                    ps = psA.tile([P, D], f32, tag="v")
                    for c0, cw in ((0, 512), (512, 256)):
                        for k in range(KC):
                            nc.tensor.matmul(
                                ps[:, c0 : c0 + cw],
                                lhsT=r(xT_sb[:, k, i * P : (i + 1) * P]),
                                rhs=r(w1T_sb[:, k, 2 * D + c0 : 2 * D + c0 + cw]),
                                start=(k == 0),
                                stop=(k == KC - 1),
                            )
                    for c in range(2):
                        nc.scalar.copy(
                            V_v[:, i, 6 * c : 6 * (c + 1), 0:HD],
                            ps[:, c * 384 : (c + 1) * 384].rearrange(
                                "p (h q) -> p h q", q=HD
                            ),
                        )

            # -------- Phase B: attention (+ prefetch of phase C inputs) -----
            with tc.tile_pool(name="late", bufs=1) as pL:
                w2Tb_sb = pL.tile([P, KC + 1, D], f32r)
                out_sb = pL.tile([P, LT, D], f32)
                nc.sync.dma_start(
                    out=w2Tb_sb[:, 0:KC, :],
                    in_=w2Tb_d[0:D].rearrange("(o p) f -> p o f", p=P),
                )
                nc.sync.dma_start(out=w2Tb_sb[0:1, KC, :], in_=w2Tb_d[D : D + 1, :])

                with tc.tile_pool(name="pt", bufs=2) as ptp, tc.tile_pool(
                    name="norm", bufs=1
                ) as pn, tc.tile_pool(name="psS", bufs=2, space="PSUM") as psS, tc.tile_pool(
                    name="psO", bufs=1, space="PSUM"
                ) as psO:
                    for t in range(NP):
                        oA = psO.tile([P, L], f32, tag="oA")
                        oB = psO.tile([P, L], f32, tag="oB")
                        otiles = (oA, oB)
                        for j in range(LT):
                            for hh in range(2):
                                h = 2 * t + hh
                                ro = 64 * hh
                                sps = psS.tile([P, L], f32, tag="s")
                                for c in range(2):
                                    nc.tensor.matmul(
                                        sps[:, c * 512 : (c + 1) * 512],
                                        lhsT=r(
                                            qkT_sb[
                                                ro : ro + 64,
                                                KC + t,
                                                j * P : (j + 1) * P,
                                            ]
                                        ),
                                        rhs=r(
                                            qkT_sb[
                                                ro : ro + 64, t, c * 512 : (c + 1) * 512
                                            ]
                                        ),
                                        start=True,
                                        stop=True,
                                    )
                                pt_t = ptp.tile([P, L], f32r, tag=f"pt{hh}")
                                nc.scalar.activation(
                                    pt_t[:],
                                    sps[:],
                                    AF.Exp,
                                    bias=bias_sb[:, j : j + 1],
                                    scale=SCALE,
                                )
                                for c in range(2):
                                    nc.tensor.matmul(
                                        otiles[hh][0:65, c * 512 : (c + 1) * 512],
                                        lhsT=r(V_v[:, j, h, :]),
                                        rhs=r(pt_t[:, c * 512 : (c + 1) * 512]),
                                        start=(j == 0),
                                        stop=(j == LT - 1),
                                    )
                        # stage O' to SBUF fast (frees the PSUM accumulators),
                        # then normalize off the critical path
                        osA = pn.tile([65, L], f32, tag="osA")
                        osB = pn.tile([65, L], f32, tag="osB")
                        nc.vector.tensor_copy(osA[:], oA[0:65, :])
                        nc.vector.tensor_copy(osB[:], oB[0:65, :])
                        # move denominator rows to physical partition 0
                        # (partition_broadcast only reads partition 0 on HW)
                        den0 = pn.tile([1, 2, L], f32, tag="den0")
                        nc.gpsimd.dma_start(out=den0[0:1, 0, :], in_=osA[64:65, :])
                        nc.gpsimd.dma_start(out=den0[0:1, 1, :], in_=osB[64:65, :])
                        denr = pn.tile([1, 2, L], f32, tag="denr")
                        nc.vector.reciprocal_approx_fast(
                            denr[0:1, :, :], den0[0:1, :, :]
                        )
                        rep = pn.tile([64, 2, L], f32, tag="rep")
                        nc.gpsimd.partition_broadcast(
                            rep[0:64, 0, :], denr[0:1, 0, :], channels=64
                        )
                        nc.gpsimd.partition_broadcast(
                            rep[0:64, 1, :], denr[0:1, 1, :], channels=64
                        )
                        btmp = pn.tile([64, L], f32r, tag="btmp")
                        nc.vector.tensor_mul(
                            OT_sb[0:64, t, :], osA[0:64, :], rep[0:64, 0, :]
                        )
                        nc.vector.tensor_mul(
                            btmp[0:64, :], osB[0:64, :], rep[0:64, 1, :]
                        )
                        nc.scalar.dma_start(out=OT_sb[64:128, t, :], in_=btmp[0:64, :])

                # ---------------- Phase C: output projection ----------------
                with tc.tile_pool(name="psC", bufs=2, space="PSUM") as psC:
                    out_r = out_d.ap().rearrange("(o p) f -> p o f", p=P)
                    for i in range(LT):
                        ps = psC.tile([P, D], f32, tag="prj")
                        for c0, cw in ((0, 512), (512, 256)):
                            for k in range(KC):
                                nc.tensor.matmul(
                                    ps[:, c0 : c0 + cw],
                                    lhsT=r(OT_sb[:, k, i * P : (i + 1) * P]),
                                    rhs=r(w2Tb_sb[:, k, c0 : c0 + cw]),
                                    start=(k == 0),
                                    stop=False,
                                )
                            # bias via ones-row rank-1 matmul
                            nc.tensor.matmul(
                                ps[:, c0 : c0 + cw],
                                lhsT=r(ones_sb[0:1, 0:P]),
                                rhs=r(w2Tb_sb[0:1, KC, c0 : c0 + cw]),
                                start=False,
                                stop=True,
                            )
                        nc.scalar.copy(out_sb[:, i, :], ps[:])
                        nc.scalar.dma_start(out=out_r[:, i, :], in_=out_sb[:, i, :])

    nc.compile()
    return nc


def _get_program():
    if "nc" not in _cached:
        _cached["nc"] = _build_program()
    return _cached["nc"]


def _prep_inputs(x, attn_mask, w_qkv, w_proj, b_proj):
    x = np.asarray(x, dtype=np.float32)
    attn_mask = np.asarray(attn_mask)
    w1T = np.ascontiguousarray(np.asarray(w_qkv, np.float32).T)        # [768, 2304]
    w2Tb = np.concatenate(
        [np.asarray(w_proj, np.float32).T, np.asarray(b_proj, np.float32)[None, :]],
        axis=0,
    )                                                                   # [769, 768]
    w2Tb = np.ascontiguousarray(w2Tb)
    in_maps = []
    for b in range(B):
        xT = np.ascontiguousarray(x[b].T)                               # [768, 1024]
        mb = NEG * (1 - attn_mask[b].astype(np.float32))                # [1024]
        in_maps.append(
            {
                "xT": xT,
                "w1T": w1T,
                "w2Tb": w2Tb,
                "mbias": mb.astype(np.float32),
                "ones": np.ones((P, H + 1), np.float32),
            }
        )
    return in_maps


def run(x, attn_mask, w_qkv, w_proj, b_proj, trace=False, **spmd_kwargs):
    from concourse.bass_utils import run_bass_kernel_spmd

    nc = _get_program()
    in_maps = _prep_inputs(x, attn_mask, w_qkv, w_proj, b_proj)
    res = run_bass_kernel_spmd(
        nc, in_maps, list(range(NCORES)), trace=trace, **spmd_kwargs
    )
    out = np.stack([np.asarray(res.results[b]["out"]) for b in range(B)], axis=0)
    return out.astype(np.float32), res


def kernel(x, attn_mask, w_qkv, w_proj, b_proj):
    out, _ = run(x, attn_mask, w_qkv, w_proj, b_proj)
    return out


# revision 14
# speedup vs baseline: 1.0213x; 1.0213x over previous
"""Fused multi-head attention layer for Trainium2, 8-core data-parallel.

Problem: x[8,1024,768] -> qkv proj (w_qkv[2304,768]) -> 12-head attention
(head_dim 64, key-padding mask) -> out proj (w_proj[768,768] + b_proj).

Strategy:
  * Data parallel over batch: core b handles x[b] end to end. No collectives.
  * Host pre-transposes x / w_qkv / w_proj so every device matmul is
    native-layout (contraction dim on partitions): xT[d,l], w1T[d,e],
    w2T[din,dout] (+bias row).
  * QK^T is computed as qkvT[e,l] (e on partitions) so per-head Q^T/K^T
    [64,1024] slices are direct matmul operands; V is computed un-transposed
    [l, 768] so V'[m, 65] slices (with a ones column) are AV lhsT operands.
  * Scores are computed TRANSPOSED: S.T[m,l] = K @ Q.T. Softmax runs without
    max-subtraction (scores are O(1) by construction: x~N(0,1), w~N(0,.02^2)),
    so exp is a single scalar-engine activation with the key-padding mask
    folded in as a per-partition additive bias and the 1/sqrt(hd) scale folded
    into the activation scale. exp(S.T) is then directly the rhs of the AV
    matmul -- no P transpose anywhere.
  * The softmax denominator comes free from a ones column appended to V
    (row 64 of the AV accumulator). O' is staged to SBUF immediately (2 DVE
    copies) so the PSUM accumulator frees fast; normalization (DVE
    reciprocal-approx + GPSIMD partition_broadcast + DVE multiply) runs off
    the critical path, overlapped with the next head pair.
  * All matmuls use float32r (full fp32 data, 1 cycle/row on TRN2 for free
    dim >= 256) -- fp32 accuracy at bf16 speed.
  * PSUM->SBUF evacuation of the projection phases runs on the scalar engine
    (idle there), keeping DVE for the attention epilogue.
"""

import os
import sys

import numpy as np

sys.path.insert(0, "/opt/trn_rl_repo")

B, L, D, H, HD = 8, 1024, 768, 12, 64
E = 3 * D
SCALE = HD ** -0.5
P = 128
KC = D // P          # 6 contraction chunks of 128 over d
LT = L // P          # 8 l/m partition tiles
NP = H // 2          # 6 head pairs
NCORES = 8
NEG = -30000.0       # mask bias; exp(NEG + s) == 0 in fp32

_cached = {}


def _build_program():
    import concourse.tile as tile
    from concourse import bacc, mybir

    f32 = mybir.dt.float32
    f32r = mybir.dt.float32r
    AF = mybir.ActivationFunctionType

    nc = bacc.Bacc(trn_type="TRN2", target_bir_lowering=False, debug=False)

    xT_d = nc.declare_dram_parameter("xT", [D, L], f32r, isOutput=False)
    w1T_d = nc.declare_dram_parameter("w1T", [D, E], f32r, isOutput=False)
    w2Tb_d = nc.declare_dram_parameter("w2Tb", [D + 1, D], f32r, isOutput=False)
    mbias_d = nc.declare_dram_parameter("mbias", [L], f32, isOutput=False)
    ones_d = nc.declare_dram_parameter("ones", [P, H + 1], f32r, isOutput=False)
    out_d = nc.declare_dram_parameter("out", [L, D], f32, isOutput=True)

    def r(ap):
        return ap

    with tile.TileContext(nc) as tc:
        from contextlib import ExitStack

        with ExitStack() as ctx:
            persist = ctx.enter_context(tc.tile_pool(name="persist", bufs=1))
            # qkvT for Q and K: e-tiles 0..5 = Q heads (2 per tile), 6..11 = K
            qkT_sb = persist.tile([P, 2 * KC, L], f32r)
            # V with a ones column per head: [l-tile, head, 65]
            V_sb = persist.tile([P, LT, H * (HD + 1)], f32r)
            V_v = V_sb[:].rearrange("p l (h c) -> p l h c", c=HD + 1)
            OT_sb = persist.tile([P, KC, L], f32r)       # O.T, heads stacked
            bias_sb = persist.tile([P, LT], f32)        # mask bias per key pos
            ones_sb = persist.tile([1, P], f32r)

            nc.sync.dma_start(
                out=ones_sb[0:1, :],
                in_=ones_d.ap().rearrange("p h -> (p h)")[None, 0:P],
            )
            for j in range(LT):
                nc.sync.dma_start(
                    out=V_v[:, j, :, HD], in_=ones_d[:, 0:H]
                )
            nc.sync.dma_start(
                out=bias_sb[:], in_=mbias_d.ap().rearrange("(o p) -> p o", p=P)
            )

            # ---------------- Phase A: QKV projection ----------------
            with tc.tile_pool(name="phA", bufs=1) as pA, tc.tile_pool(
                name="psA", bufs=2, space="PSUM"
            ) as psA:
                xT_sb = pA.tile([P, KC, L], f32r)
                w1T_sb = pA.tile([P, KC, E], f32r)
                xT_r = xT_d.ap().rearrange("(o p) l -> p o l", p=P)
                w1T_r = w1T_d.ap().rearrange("(o p) e -> p o e", p=P)
                # chunked loads so the first matmuls start early
                for k in range(KC):
                    nc.sync.dma_start(out=xT_sb[:, k, :], in_=xT_r[:, k, :])
                EW = 256
                for e0 in range(0, E, EW):
                    nc.sync.dma_start(
                        out=w1T_sb[:, :, e0 : e0 + EW],
                        in_=w1T_r[:, :, e0 : e0 + EW],
                    )

                # qkT[e,l] = w1.T.T @ xT for e in [0, 1536)
                for et in range(2 * KC):
                    ps = psA.tile([P, L], f32, tag="qk")
                    for c in range(2):
                        for k in range(KC):
                            nc.tensor.matmul(
                                ps[:, c * 512 : (c + 1) * 512],
                                lhsT=r(w1T_sb[:, k, et * P : (et + 1) * P]),
                                rhs=r(xT_sb[:, k, c * 512 : (c + 1) * 512]),
                                start=(k == 0),
                                stop=(k == KC - 1),
                            )
                    nc.scalar.copy(qkT_sb[:, et, :], ps[:])

                # V[l, dv] = x @ w1_v.T  (dv in [1536, 2304))
                for i in range(LT):
                    ps = psA.tile([P, D], f32, tag="v")
                    for c0, cw in ((0, 512), (512, 256)):
                        for k in range(KC):
                            nc.tensor.matmul(
                                ps[:, c0 : c0 + cw],
                                lhsT=r(xT_sb[:, k, i * P : (i + 1) * P]),
                                rhs=r(w1T_sb[:, k, 2 * D + c0 : 2 * D + c0 + cw]),
                                start=(k == 0),
                                stop=(k == KC - 1),
                            )
                    for c in range(2):
                        nc.scalar.copy(
                            V_v[:, i, 6 * c : 6 * (c + 1), 0:HD],
                            ps[:, c * 384 : (c + 1) * 384].rearrange(
                                "p (h q) -> p h q", q=HD
                            ),
                        )

            # -------- Phase B: attention (+ prefetch of phase C inputs) -----
            with tc.tile_pool(name="late", bufs=1) as pL:
                w2Tb_sb = pL.tile([P, KC + 1, D], f32r)
                out_sb = pL.tile([P, LT, D], f32)
                nc.sync.dma_start(
                    out=w2Tb_sb[:, 0:KC, :],
                    in_=w2Tb_d[0:D].rearrange("(o p) f -> p o f", p=P),
                )
                nc.sync.dma_start(out=w2Tb_sb[0:1, KC, :], in_=w2Tb_d[D : D + 1, :])

                with tc.tile_pool(name="pt", bufs=2) as ptp, tc.tile_pool(
                    name="norm", bufs=1
                ) as pn, tc.tile_pool(name="psS", bufs=2, space="PSUM") as psS, tc.tile_pool(
                    name="psO", bufs=1, space="PSUM"
                ) as psO:
                    for t in range(NP):
                        oA = psO.tile([P, L], f32, tag="oA")
                        oB = psO.tile([P, L], f32, tag="oB")
                        otiles = (oA, oB)
                        for j in range(LT):
                            for hh in range(2):
                                h = 2 * t + hh
                                ro = 64 * hh
                                sps = psS.tile([P, L], f32, tag="s")
                                for c in range(2):
                                    nc.tensor.matmul(
                                        sps[:, c * 512 : (c + 1) * 512],
                                        lhsT=r(
                                            qkT_sb[
                                                ro : ro + 64,
                                                KC + t,
                                                j * P : (j + 1) * P,
                                            ]
                                        ),
                                        rhs=r(
                                            qkT_sb[
                                                ro : ro + 64, t, c * 512 : (c + 1) * 512
                                            ]
                                        ),
                                        start=True,
                                        stop=True,
                                    )
                                pt_t = ptp.tile([P, L], f32r, tag=f"pt{hh}")
                                nc.scalar.activation(
                                    pt_t[:],
                                    sps[:],
                                    AF.Exp,
                                    bias=bias_sb[:, j : j + 1],
                                    scale=SCALE,
                                )
                                for c in range(2):
                                    nc.tensor.matmul(
                                        otiles[hh][0:65, c * 512 : (c + 1) * 512],
                                        lhsT=r(V_v[:, j, h, :]),
                                        rhs=r(pt_t[:, c * 512 : (c + 1) * 512]),
                                        start=(j == 0),
                                        stop=(j == LT - 1),
                                    )
                        # stage O' to SBUF fast (frees the PSUM accumulators),
                        # then normalize off the critical path
                        osA = pn.tile([65, L], f32, tag="osA")
                        osB = pn.tile([65, L], f32, tag="osB")
                        nc.vector.tensor_copy(osA[:], oA[0:65, :])
                        nc.vector.tensor_copy(osB[:], oB[0:65, :])
                        # move denominator rows to physical partition 0
                        # (partition_broadcast only reads partition 0 on HW)
                        den0 = pn.tile([1, 2, L], f32, tag="den0")
                        nc.sync.dma_start(out=den0[0:1, 0, :], in_=osA[64:65, :])
                        nc.sync.dma_start(out=den0[0:1, 1, :], in_=osB[64:65, :])
                        denr = pn.tile([1, 2, L], f32, tag="denr")
                        nc.vector.reciprocal_approx_fast(
                            denr[0:1, :, :], den0[0:1, :, :]
                        )
                        rep = pn.tile([64, 2, L], f32, tag="rep")
                        nc.gpsimd.partition_broadcast(
                            rep[0:64, 0, :], denr[0:1, 0, :], channels=64
                        )
                        nc.gpsimd.partition_broadcast(
                            rep[0:64, 1, :], denr[0:1, 1, :], channels=64
                        )
                        btmp = pn.tile([64, L], f32r, tag="btmp")
                        nc.vector.tensor_mul(
                            OT_sb[0:64, t, :], osA[0:64, :], rep[0:64, 0, :]
                        )
                        nc.vector.tensor_mul(
                            btmp[0:64, :], osB[0:64, :], rep[0:64, 1, :]
                        )
                        nc.sync.dma_start(out=OT_sb[64:128, t, :], in_=btmp[0:64, :])

                # ---------------- Phase C: output projection ----------------
                with tc.tile_pool(name="psC", bufs=2, space="PSUM") as psC:
                    out_r = out_d.ap().rearrange("(o p) f -> p o f", p=P)
                    for i in range(LT):
                        ps = psC.tile([P, D], f32, tag="prj")
                        for c0, cw in ((0, 512), (512, 256)):
                            for k in range(KC):
                                nc.tensor.matmul(
                                    ps[:, c0 : c0 + cw],
                                    lhsT=r(OT_sb[:, k, i * P : (i + 1) * P]),
                                    rhs=r(w2Tb_sb[:, k, c0 : c0 + cw]),
                                    start=(k == 0),
                                    stop=False,
                                )
                            # bias via ones-row rank-1 matmul
                            nc.tensor.matmul(
                                ps[:, c0 : c0 + cw],
                                lhsT=r(ones_sb[0:1, 0:P]),
                                rhs=r(w2Tb_sb[0:1, KC, c0 : c0 + cw]),
                                start=False,
                                stop=True,
                            )
                        nc.scalar.copy(out_sb[:, i, :], ps[:])
                        nc.sync.dma_start(out=out_r[:, i, :], in_=out_sb[:, i, :])

    nc.compile()
    return nc


def _get_program():
    if "nc" not in _cached:
        _cached["nc"] = _build_program()
    return _cached["nc"]


def _prep_inputs(x, attn_mask, w_qkv, w_proj, b_proj):
    x = np.asarray(x, dtype=np.float32)
    attn_mask = np.asarray(attn_mask)
    w1T = np.ascontiguousarray(np.asarray(w_qkv, np.float32).T)        # [768, 2304]
    w2Tb = np.concatenate(
        [np.asarray(w_proj, np.float32).T, np.asarray(b_proj, np.float32)[None, :]],
        axis=0,
    )                                                                   # [769, 768]
    w2Tb = np.ascontiguousarray(w2Tb)
    in_maps = []
    for b in range(B):
        xT = np.ascontiguousarray(x[b].T)                               # [768, 1024]
        mb = NEG * (1 - attn_mask[b].astype(np.float32))                # [1024]
        in_maps.append(
            {
                "xT": xT,
                "w1T": w1T,
                "w2Tb": w2Tb,
                "mbias": mb.astype(np.float32),
                "ones": np.ones((P, H + 1), np.float32),
            }
        )
    return in_maps


def run(x, attn_mask, w_qkv, w_proj, b_proj, trace=False, **spmd_kwargs):
    from concourse.bass_utils import run_bass_kernel_spmd

    nc = _get_program()
    in_maps = _prep_inputs(x, attn_mask, w_qkv, w_proj, b_proj)
    res = run_bass_kernel_spmd(
        nc, in_maps, list(range(NCORES)), trace=trace, **spmd_kwargs
    )
    out = np.stack([np.asarray(res.results[b]["out"]) for b in range(B)], axis=0)
    return out.astype(np.float32), res


def kernel(x, attn_mask, w_qkv, w_proj, b_proj):
    out, _ = run(x, attn_mask, w_qkv, w_proj, b_proj)
    return out
